# revision 50
# baseline (speedup 1.0000x reference)
"""Trainium2 Bass kernel for nn_NetworkLayer_79173427134941 (gnn_message_passing).

Reference computation (per batch item b, N=1024 points, 3D coords):
    norms = ||x_b||                      [N, 1]
    dots  = sqrt(x_b @ x_b^T)            [N, N]
    scalars = [u_b (G=8) | norms | dots] [N, 1033]
    h = LeakyReLU(scalars @ W0 + b0); h = LeakyReLU(h @ W1 + b1)
    fk = h @ W2 + b2                     [N, 128]
    out_b = einsum('io,id->od', fk, x_b) / N    [128, 3]

Strategy:
  - Data-parallel over batch: 4 batch items per core x 8 cores.
  - Never materialize dots in HBM: gram on TensorE (fp32r), sqrt on ScalarE
    during PSUM->SBUF eviction, MLP fused on-chip in transposed [H, N] layout.
  - u-part + b0 folded into a host-precomputed K=2 rhs chunk [norms; ones].
  - Final contraction uses associativity:
       out_b^T = (x_b^T @ h1) @ W2 + b2 (x) colsum(x_b)
    so the device only returns y_b = x_b^T @ h1  [3, 128]; the last tiny
    [3,128]@[128,128] matmul + bias outer product runs on host.
"""

import numpy as np

B, N, G = 32, 1024, 8
H, K_OUT = 128, 128
N_CORES = 8
BPC = B // N_CORES  # batch items per core

_cached = {}
# "f32r": all matmuls fp32r (max accuracy). "f16": dots + MLP tail in fp16
# (1 cyc/row at any moving size + fast weight loads, ~5e-4 quantization).
PRECISION = "f16"
# PE row-tiling of the gram was tried and abandoned: the row-group matmuls
# (32x128 array mode) interleave with 128-row MLP matmuls, and the required
# array-mode-switch drains are not emitted by this toolchain (fp32r weights
# even fault the exec unit; fp16 silently corrupts the odd strips).
ROWTILE_GRAM = False



def _build_nc(precision=None, repeat=1, with_b1=True):
    import concourse.tile as tile
    from concourse import bacc, mybir

    precision = precision or PRECISION
    f32 = mybir.dt.float32
    f32r = mybir.dt.float32r
    f16 = mybir.dt.float16
    tdt = f16 if precision == "f16" else f32r   # tail: h0/h1c/w1/xc
    mdt = f16 if precision == "f16" else f32r   # mid: dots/w0d
    MUL = mybir.AluOpType.mult
    ADD = mybir.AluOpType.add

    nc = bacc.Bacc(
        "TRN2",
        target_bir_lowering=False,
        debug=False,
        enable_asserts=True,
        num_devices=N_CORES,
    )

    # DRAM I/O (per core)
    gdt = tdt  # gram operand dtype (fp16: fast weight loads, no fp32 self-load)
    xT_d = nc.dram_tensor("xT", [BPC, 3, N], gdt, kind="ExternalInput").ap()
    xbc_d = nc.dram_tensor("xbc", [BPC, 128, 3 * N], tdt, kind="ExternalInput").ap()
    rhs2_d = nc.dram_tensor("rhs2", [BPC, 2, N], tdt, kind="ExternalInput").ap()
    lw2_d = nc.dram_tensor("lw2", [BPC, 2, H], tdt, kind="ExternalInput").ap()
    w0d_d = nc.dram_tensor("w0d", [128, 1024], mdt, kind="ExternalInput").ap()
    w1_d = nc.dram_tensor("w1", [128, H], tdt, kind="ExternalInput").ap()
    b1t_d = nc.dram_tensor("b1t", [1, N], tdt, kind="ExternalInput").ap()
    ones_d = nc.dram_tensor("ones", [1, N], tdt, kind="ExternalInput").ap()
    y_d = nc.dram_tensor("y", [BPC, H, 3], f32, kind="ExternalOutput").ap()

    NCHUNK = N // 128  # 8 K-chunks of the dots matmul

    with tile.TileContext(nc) as tc:
        with (
            tc.tile_pool(name="const", bufs=1) as constp,
            tc.tile_pool(name="data", bufs=2) as datap,
            tc.tile_pool(name="dots", bufs=2) as dotsp,
            tc.tile_pool(name="act", bufs=2) as actp,
            tc.tile_pool(name="yout", bufs=2) as youtp,
            tc.tile_pool(name="gram", bufs=2, space="PSUM") as gramp,
            tc.tile_pool(name="h0p", bufs=1, space="PSUM") as h0pp,
            tc.tile_pool(name="h1p", bufs=1, space="PSUM") as h1pp,
        ):
            # const tiles (DMAs issued after batch-0 loads; see emit_consts)
            w0d_sb = constp.tile([128, 1024], mdt)
            w1_sb = constp.tile([128, H], tdt)
            b1t_sb = constp.tile([1, N], tdt)
            ones_sb = constp.tile([1, N], tdt)

            def emit_consts():
                nc.sync.dma_start(out=w0d_sb[:], in_=w0d_d[:])
                nc.sync.dma_start(out=w1_sb[:], in_=w1_d[:])
                nc.sync.dma_start(out=b1t_sb[:], in_=b1t_d[:])
                nc.sync.dma_start(out=ones_sb[:], in_=ones_d[:])

            def leaky_evict(out_ap, ps_ap, tmp_ap, use_act=False):
                # leaky(x) = 0.01*x + 0.99*relu(x); two ops so each reads PSUM once.
                # use_act routes the relu-scale half to ScalarE (only worth it
                # for the last batch item, when the sqrt stream has drained).
                if use_act:
                    nc.scalar.activation(
                        tmp_ap, ps_ap, mybir.ActivationFunctionType.Relu,
                        bias=0.0, scale=0.99,
                    )
                else:
                    nc.vector.tensor_scalar(
                        tmp_ap, ps_ap, 0.0, 0.99, mybir.AluOpType.max, MUL
                    )
                nc.vector.scalar_tensor_tensor(out_ap, ps_ap, 0.01, tmp_ap, MUL, ADD)

            def emit_gram_strip(b, m, st):
                """Gram strip m of batch b: 2 matmuls + sqrt eviction."""
                if m == 0:
                    xt_parts = 35 if ROWTILE_GRAM else 3
                    xT_sb = datap.tile([xt_parts, N], gdt, tag="xT", name=f"xT{b}")
                    nc.sync.dma_start(out=xT_sb[0:3, :], in_=xT_d[b])
                    if ROWTILE_GRAM:
                        nc.sync.dma_start(out=xT_sb[32:35, :], in_=xT_d[b])
                    rhs2_sb = datap.tile([2, N], tdt, tag="rhs2", name=f"rhs2{b}")
                    nc.sync.dma_start(out=rhs2_sb[:], in_=rhs2_d[b])
                    lw2_sb = datap.tile([2, H], tdt, tag="lw2", name=f"lw2{b}")
                    nc.sync.dma_start(out=lw2_sb[:], in_=lw2_d[b])
                    if b == 0:
                        emit_consts()
                    dots_sb = dotsp.tile([128, NCHUNK * N], mdt, tag="dots",
                                         name=f"dots{b}")
                    # x^T rows pre-broadcast across partitions on the host;
                    # consumed by the DVE y-reduction
                    xbc_sb = datap.tile([128, 3 * N], tdt, tag="xbc",
                                        name=f"xbc{b}")
                    nc.sync.dma_start(out=xbc_sb[:], in_=xbc_d[b])
                    st.update(xT=xT_sb, xbc=xbc_sb, rhs2=rhs2_sb, lw2=lw2_sb,
                              dots=dots_sb)
                xT_sb, dots_sb = st["xT"], st["dots"]
                g_ps = gramp.tile([128, N], f32, tag="g", name=f"g{b}_{m}")
                lhsT = xT_sb[:, 128 * m : 128 * (m + 1)]
                for half in range(2):
                    nc.tensor.matmul(
                        g_ps[:, 512 * half : 512 * (half + 1)],
                        lhsT,
                        xT_sb[:, 512 * half : 512 * (half + 1)],
                        start=True,
                        stop=True,
                    )
                nc.scalar.sqrt(dots_sb[:, N * m : N * (m + 1)], g_ps[:])

            def emit_h0_chunk(b, c, st):
                """Layer-0 K-chunk c of batch b (needs dots strip c only)."""
                if c == 0:
                    h0_ps = h0pp.tile([128, N], f32, tag="h0ps", name=f"h0ps{b}")
                    st["h0ps"] = h0_ps
                    for half in range(2):
                        sl = slice(512 * half, 512 * (half + 1))
                        nc.tensor.matmul(
                            h0_ps[:, sl],
                            st["lw2"][:],
                            st["rhs2"][:, sl],
                            start=True,
                            stop=False,
                        )
                h0_ps, dots_sb = st["h0ps"], st["dots"]
                lhsT = w0d_sb[:, 128 * c : 128 * (c + 1)]
                for half in range(2):
                    nc.tensor.matmul(
                        h0_ps[:, 512 * half : 512 * (half + 1)],
                        lhsT,
                        dots_sb[:, N * c + 512 * half : N * c + 512 * (half + 1)],
                        start=False,
                        stop=(c == NCHUNK - 1),
                    )
                if c == NCHUNK - 1:
                    h0_sb = actp.tile([128, N], tdt, tag="h0", name=f"h0{b}")
                    st["h0"] = h0_sb
                    for half in range(2):
                        sl = slice(512 * half, 512 * (half + 1))
                        ltmp = actp.tile([128, 512], f32, tag="ltmp", bufs=4,
                                         name=f"ltmp0_{b}_{half}")
                        leaky_evict(h0_sb[:, sl], h0_ps[:, sl], ltmp[:],
                                    use_act=(b == BPC - 1))

            def emit_tail(b, st):
                """Layer 1 (transposed [H, N] layout) + output contraction."""
                h0_sb, xbc_sb = st["h0"], st["xbc"]
                h1_ps = h1pp.tile([128, N], f32, tag="h1ps", name=f"h1ps{b}")
                for half in range(2):
                    sl = slice(512 * half, 512 * (half + 1))
                    if with_b1:
                        # bias as a rank-1 matmul b1 (x) ones; skipped when
                        # the host sees b1 == 0 (true for this problem)
                        nc.tensor.matmul(
                            h1_ps[:, sl],
                            b1t_sb[:, 0:128],
                            ones_sb[:, sl],
                            start=True,
                            stop=False,
                        )
                    nc.tensor.matmul(
                        h1_ps[:, sl],
                        w1_sb[:],
                        h0_sb[:, sl],
                        start=not with_b1,
                        stop=True,
                    )
                h1c_sb = actp.tile([128, N], tdt, tag="h1c", name=f"h1c{b}")
                for half in range(2):
                    sl = slice(512 * half, 512 * (half + 1))
                    ltmp1 = actp.tile([128, 512], f32, tag="ltmp", bufs=4,
                                      name=f"ltmp1_{b}_{half}")
                    leaky_evict(h1c_sb[:, sl], h1_ps[:, sl], ltmp1[:],
                                use_act=(b == BPC - 1))

                # y_b^T[h, d] = sum_i h1^T[h, i] * x[i, d]: free-axis
                # multiply-reduce on DVE against the broadcast x rows
                yT_sb = youtp.tile([128, 4], f32, tag="y", name=f"y{b}")
                for d in range(3):
                    ysc = actp.tile([128, N], tdt, tag="ysc", name=f"ysc{b}_{d}")
                    nc.vector.scalar_tensor_tensor(
                        ysc[:],
                        h1c_sb[:],
                        1.0,
                        xbc_sb[:, N * d : N * (d + 1)],
                        MUL,
                        MUL,
                        accum_out=yT_sb[:, d : d + 1],
                    )
                nc.sync.dma_start(out=y_d[b], in_=yT_sb[:, 0:3])

            # Software-pipelined emission, one stage per batch item:
            #   [gram strips b] [tail of b-1] [h0 chunks of b]
            # Priorities follow emission order, so the previous item's
            # MLP tail fills TensorE while ScalarE streams this item's
            # sqrts; h0 chunk c only needs sqrt strip c, so the h0 block
            # drains right behind the sqrt stream.
            def emit_all():
                states = [dict() for _ in range(BPC)]
                for b in range(BPC):
                    for m in range(NCHUNK):
                        emit_gram_strip(b, m, states[b])
                    if b >= 1:
                        emit_tail(b - 1, states[b - 1])
                    for c in range(NCHUNK):
                        emit_h0_chunk(b, c, states[b])
                emit_tail(BPC - 1, states[BPC - 1])

            if repeat == 1:
                emit_all()
            else:
                # benchmark mode: repeat the whole (idempotent) pipeline so
                # device time dominates host/tunnel dispatch overhead
                with tc.For_i(0, repeat, 1):
                    emit_all()

    nc.finalize()
    return nc


def _host_prep(x, u, W0, b0, W1, b1):
    """Build per-core input maps."""
    tnp = np.float16 if PRECISION == "f16" else np.float32
    gnp = tnp
    xT = np.ascontiguousarray(x.transpose(0, 2, 1)).astype(gnp)  # [B, 3, N]
    # [B, 128, 3N]: row d of x^T broadcast across the partition dim
    xbc = np.ascontiguousarray(
        np.broadcast_to(xT.reshape(B, 1, 3 * N), (B, 128, 3 * N))
    )
    norms = np.sqrt((x.astype(np.float64) ** 2).sum(-1)).astype(np.float32)  # [B, N]
    rhs2 = np.stack([norms, np.ones_like(norms)], axis=1)  # [B, 2, N]
    cb = (u @ W0[:G] + b0).astype(np.float32)  # [B, H]
    w0n = np.broadcast_to(W0[G], (B, H)).astype(np.float32)
    lw2 = np.ascontiguousarray(np.stack([w0n, cb], axis=1))  # [B, 2, H]
    w0d = np.ascontiguousarray(
        W0[G + 1 :].reshape(N // 128, 128, H).transpose(1, 0, 2).reshape(128, N // 128 * H)
    )

    in_maps = []
    for c in range(N_CORES):
        sl = slice(BPC * c, BPC * (c + 1))
        in_maps.append(
            {
                "xT": np.ascontiguousarray(xT[sl]),
                "xbc": np.ascontiguousarray(xbc[sl]),
                "rhs2": np.ascontiguousarray(rhs2[sl]).astype(tnp),
                "lw2": np.ascontiguousarray(lw2[sl]).astype(tnp),
                "w0d": w0d.astype(tnp),
                "w1": np.ascontiguousarray(W1).astype(tnp),
                "b1t": np.tile(b1, N // H)[None, :].astype(tnp),
                "ones": np.ones((1, N), dtype=tnp),
            }
        )
    return in_maps


def kernel(x, u, W0, b0, W1, b1, W2, b2, _run_kwargs=None):
    x = np.asarray(x, dtype=np.float32)
    u = np.asarray(u, dtype=np.float32)
    W0 = np.asarray(W0, dtype=np.float32)
    b0 = np.asarray(b0, dtype=np.float32)
    W1 = np.asarray(W1, dtype=np.float32)
    b1 = np.asarray(b1, dtype=np.float32)
    W2 = np.asarray(W2, dtype=np.float32)
    b2 = np.asarray(b2, dtype=np.float32)

    from concourse.bass_utils import run_bass_kernel_spmd

    with_b1 = bool(np.any(b1))
    key = ("nc", with_b1)
    if key not in _cached:
        _cached[key] = _build_nc(with_b1=with_b1)
    nc = _cached[key]

    in_maps = _host_prep(x, u, W0, b0, W1, b1)
    kw = dict(_run_kwargs or {})
    res = run_bass_kernel_spmd(nc, in_maps, list(range(N_CORES)), **kw)
    _cached["last_results"] = res
    y = np.concatenate([r["y"] for r in res.results], axis=0)  # [B, H, 3]

    # host finish: out[b,o,d] = sum_h W2[h,o] y[b,h,d] / N + b2[o]*colsum_x[b,d]/N
    colsum = x.sum(axis=1)  # [B, 3]
    out = (
        np.einsum("ho,bhd->bod", W2.astype(np.float64), y.astype(np.float64))
        + b2.astype(np.float64)[None, :, None] * colsum.astype(np.float64)[:, None, :]
    ) / N
    return out.astype(np.float32)
